# revision 28
# baseline (speedup 1.0000x reference)
"""Trainium2 Bass kernel for nn_GATv2_23278722744604.

2-layer GATv2 (N=50000 nodes, E=800000 edges, 128 feats, 4 heads x 32).
Sharding: destination-node blocks across 8 NeuronCores; edges routed to the
owner of their dst node.

v2 design (per layer, per core):
- project features into DRAM tables (row-permuted so table writes are
  contiguous per partition);
- per 4096-edge chunk, dma_gather with transpose=True fetches projected
  src/dst rows in FEATURE-major layout [128f, E];
- u = fsT + fdT (DVE), lr = prelu(u) (Act), logits = attn-block matmul on PE
  (contraction over the feature partition dim), ex = exp (Act);
- PE transposes bring fs and ex back to edge-major; messages m = ex * fs;
- one-hot G built by is_equal (iota vs dup-pair dstloc), scatter-add via
  G^T @ [m | exdup] matmuls accumulated in PSUM per dst block;
- finalize: out = num/den + (residual + biases), layer0 double-elu /
  layer1 head-mean.
"""
import os
import time
import numpy as np
import ml_dtypes
import jax
from jax.sharding import Mesh, PartitionSpec, NamedSharding
from jax.experimental.shard_map import shard_map
import concourse.bass as bass
import concourse.bacc as bacc
import concourse.mybir as mybir
import concourse.tile as tile
from concourse import library_config, bass2jax
from concourse.bass2jax import _bass_exec_p, install_neuronx_cc_hook
from contextlib import ExitStack

bf16 = ml_dtypes.bfloat16
f32 = np.float32
dt = mybir.dt
A = mybir.ActivationFunctionType
O = mybir.AluOpType
SKIP = set()

N = 50000
D = 128
HEADS = 4
OUT = 32
N_CORES = 8
OWN = N // N_CORES            # 6250
NBLK = (OWN + 127) // 128     # 49
NCH_SRC = (N + 127) // 128    # 391
NPAD_SRC = 128 * NCH_SRC      # 50048
NCH_DST = NBLK                # 49
NPAD_DST = 128 * NCH_DST      # 6272
BUCKET = 32768
GCALL = 8192                  # idxs per dma_gather call
TPC = GCALL // 128            # 32 tiles per chunk
CB = 8                        # tiles per batch
SLOPE = 0.2


def srcrow(n):
    """Permuted table row for global node n (partition-major layout)."""
    return (n % 128) * NCH_SRC + n // 128


def dstrow(dl):
    return (dl % 128) * NCH_DST + dl // 128


FLEX0 = NPAD_SRC - BUCKET     # 17280: bucket1 table base (buckets overlap)


def plan_core_groups(src, dst, core):
    """Per-block edge lists for one core (permuted src rows, sorted)."""
    base = core * OWN
    sel = (dst >= base) & (dst < base + OWN)
    es = src[sel].astype(np.int64)
    ed = (dst[sel] - base).astype(np.int64)
    rs = (es % 128) * NCH_SRC + es // 128
    blks = {}
    for k in range(NBLK):
        m = ed // 128 == k
        r, sl = rs[m], ed[m] % 128
        o = np.argsort(r, kind="stable")   # rows ascending: must0,flex,must1
        blks[k] = (r[o], sl[o])
    return blks


def make_plans(src, dst):
    """Common-structure plans for all cores (same NEFF across cores).

    src-row buckets overlap: bucket0 = rows [0, BUCKET), bucket1 = rows
    [FLEX0, NPAD_SRC). Rows in [FLEX0, BUCKET) may go to either bucket, which
    lets every core fill its bucket0 group for block k with exactly
    128*m[k] edges (zero padding there).
    """
    all_blks = [plan_core_groups(src, dst, c) for c in range(N_CORES)]
    ntiles = {}
    for k in range(NBLK):
        elig0 = [int(np.searchsorted(b[k][0], BUCKET)) for b in all_blks]
        must0 = [int(np.searchsorted(b[k][0], FLEX0)) for b in all_blks]
        m_k = min(e // 128 for e in elig0)
        assert m_k * 128 >= max(must0), (k, m_k, max(must0))
        n_k = max((len(b[k][0]) - m_k * 128 + 127) // 128 for b in all_blks)
        ntiles[(0, k)] = m_k
        ntiles[(1, k)] = n_k
    m0fill = {k: ntiles[(0, k)] for k in range(NBLK)}
    run_lens = [sum(ntiles[(b, k)] for k in range(NBLK)) for b in range(2)]
    for b in range(2):
        pad = (-run_lens[b]) % TPC
        ntiles[(b, NBLK - 1)] += pad
        run_lens[b] += pad
    blocks, start, end = [], [], []
    for b in range(2):
        for k in range(NBLK):
            n = ntiles[(b, k)]
            if n == 0:
                continue
            blocks += [k] * n
            start += [True] + [False] * (n - 1)
            end += [False] * (n - 1) + [True]
    T = len(blocks)
    n_run0 = run_lens[0]
    common = dict(T=T, n_run0=n_run0, blocks=blocks,
                  start=np.array(start), end=np.array(end))

    plans = []
    for c in range(N_CORES):
        blks = all_blks[c]
        idx_src, dstloc_cols = [], []
        for b in range(2):
            for k in range(NBLK):
                n = ntiles[(b, k)]
                if n == 0:
                    continue
                rows, slots = blks[k]
                s0 = m0fill[k] * 128                 # bucket0 edge count
                if b == 0:
                    e_s, e_sl = rows[:s0], slots[:s0]
                else:
                    e_s, e_sl = rows[s0:] - FLEX0, slots[s0:]
                cap = n * 128
                pad = cap - len(e_s)
                assert pad >= 0, (b, k, n, len(e_s))
                e_s = np.concatenate([e_s, np.zeros(pad, np.int64)])
                e_sl = np.concatenate([e_sl, np.full(pad, 255, np.int64)])
                idx_src.append(e_s)
                dstloc_cols.append(e_sl.reshape(n, 128))
        p = dict(common)
        p["idx_src"] = np.concatenate(idx_src).astype(np.int16)
        dl = np.concatenate(dstloc_cols, axis=0)          # [T, 128] slots
        # transposed + duplicated pairs: [128, T, 2] -> [128, 2T]
        dlT = dl.T.astype(np.float32).astype(bf16)         # [128, T]
        p["dstloc2"] = np.repeat(dlT, 2, axis=1)           # [128, 2T]
        p["own_base"] = c * OWN
        plans.append(p)
    return plans


def build_layer(plan, layer):
    """Build the per-core NEFF for one GATv2 layer given the edge plan."""
    T = plan["T"]
    blocks, tstart, tend = plan["blocks"], plan["start"], plan["end"]
    n_run0 = plan["n_run0"]
    NIDX = T * 128
    NCHUNK = NIDX // GCALL
    chunks0 = n_run0 * 128 // GCALL   # chunks in bucket0 run

    nc = bacc.Bacc("TRN2", target_bir_lowering=False, debug=False,
                   num_devices=N_CORES)
    # projected tables are computed on host; rows permuted r = (n%128)*NCH + n//128
    tbl_src_d = nc.dram_tensor("tbl_src", (NPAD_SRC, 128), dt.bfloat16, kind="ExternalInput")
    tbl_dst_d = nc.dram_tensor("tbl_dst", (NPAD_DST, 128), dt.bfloat16, kind="ExternalInput")
    # residual+bias input and output, both in permuted row order r = p*NBLK + c
    fo_d = nc.dram_tensor("fo", (NPAD_DST, 128), dt.float32, kind="ExternalInput")
    ident_d = nc.dram_tensor("ident", (128, 128), dt.bfloat16, kind="ExternalInput")
    attnA_d = nc.dram_tensor("attnA", (128, 4), dt.bfloat16, kind="ExternalInput")
    iota_d = nc.dram_tensor("iota", (128, 128), dt.bfloat16, kind="ExternalInput")
    dstloc2_d = nc.dram_tensor("dstloc2", (128, 2 * T), dt.bfloat16, kind="ExternalInput")
    idx_src_d = nc.dram_tensor("idx_src", (128, NIDX // 16), dt.int16, kind="ExternalInput")
    OUTW = 128 if layer == 0 else OUT
    out_d = nc.dram_tensor("out_own", (NPAD_DST, OUTW), dt.float32, kind="ExternalOutput")

    with tile.TileContext(nc) as tc, ExitStack() as ctx:
        cpool = ctx.enter_context(tc.tile_pool(name="const", bufs=1))
        ipool = ctx.enter_context(tc.tile_pool(name="idx", bufs=2))
        gpool = ctx.enter_context(tc.tile_pool(name="gath", bufs=2))
        upool = ctx.enter_context(tc.tile_pool(name="u", bufs=2))
        mpool = ctx.enter_context(tc.tile_pool(name="msg", bufs=3))
        spool = ctx.enter_context(tc.tile_pool(name="scratch", bufs=3))
        apool = ctx.enter_context(tc.tile_pool(name="acc", bufs=1))
        ppool = ctx.enter_context(tc.tile_pool(name="psagg", bufs=2, space="PSUM"))
        ptpool = ctx.enter_context(tc.tile_pool(name="psT", bufs=1, space="PSUM"))
        pgpool = ctx.enter_context(tc.tile_pool(name="psGT", bufs=2, space="PSUM"))
        pupool = ctx.enter_context(tc.tile_pool(name="psu", bufs=1, space="PSUM"))
        pepool = ctx.enter_context(tc.tile_pool(name="psex", bufs=1, space="PSUM"))
        fpool = ctx.enter_context(tc.tile_pool(name="fin", bufs=1))
        f1pool = ctx.enter_context(tc.tile_pool(name="fin1", bufs=1))

        nc.gpsimd.load_library(library_config.mlp)

        # ---------------- constants ----------------
        ident_sb = cpool.tile([128, 128], dt.bfloat16)
        attnA_sb = cpool.tile([128, 4], dt.bfloat16)
        iota_sb = cpool.tile([128, 128], dt.bfloat16)
        dstloc_sb = cpool.tile([128, 2 * T], dt.bfloat16)
        nc.sync.dma_start(ident_sb[:], ident_d[:])
        nc.sync.dma_start(attnA_sb[:], attnA_d[:])
        nc.sync.dma_start(iota_sb[:], iota_d[:])
        nc.sync.dma_start(dstloc_sb[:], dstloc2_d[:])
        fdblk_sb = cpool.tile([128, NBLK, 128], dt.bfloat16)
        nc.sync.dma_start(fdblk_sb[:],
                          tbl_dst_d[:].rearrange("(p c) d -> p c d", c=NBLK))


        # ---------------- edge phase ----------------
        acc = apool.tile([128, NBLK * 136], dt.float32)
        nc.vector.memset(acc[:], 0.0)
        negone = cpool.tile([128, 1], dt.float32)
        nc.vector.memset(negone[:], -1.0)

        ps_cur = None
        IW = GCALL // 16
        for ch in range(NCHUNK):
            fsT = gpool.tile([128, 1, GCALL], dt.bfloat16, tag="fsT")
            isrc = ipool.tile([128, IW], dt.int16, tag="isrc")
            nc.sync.dma_start(isrc[:], idx_src_d[:, ch * IW:(ch + 1) * IW])
            if ch < chunks0:
                src_tab = tbl_src_d[0:BUCKET, :]
            else:
                src_tab = tbl_src_d[FLEX0:NPAD_SRC, :]
            if "gather" not in SKIP:
                nc.gpsimd.dma_gather(
                    out_ap=fsT[:], in_ap=src_tab,
                    idxs_ap=isrc[:],
                    num_idxs=GCALL, num_idxs_reg=GCALL, elem_size=128,
                    transpose=True, single_packet=False)
            else:
                nc.vector.memset(fsT[:], 0.5)

            for sb in range(TPC // CB):
                t0 = ch * TPC + sb * CB
                E0 = sb * CB * 128
                esl = slice(E0, E0 + CB * 128)
                # one-hot G for this batch (also used for fd selection)
                G = spool.tile([128, CB, 128], dt.bfloat16, tag="G")
                if "dve" not in SKIP:
                    g4 = G[:].rearrange("p c (m x) -> p c m x", x=2)
                    io2 = iota_sb[:].rearrange("p (m x) -> p () m x", x=2)
                    dl2 = dstloc_sb[:, 2 * t0:2 * (t0 + CB)].rearrange(
                        "p (c x) -> p c () x", x=2)
                    nc.vector.tensor_tensor(
                        out=g4, in0=io2.broadcast_to((128, CB, 64, 2)),
                        in1=dl2.broadcast_to((128, CB, 64, 2)), op=O.is_equal)
                # GT = transpose(G) -> SBUF (matmul rhs)
                psGT = pgpool.tile([128, CB, 128], dt.bfloat16, space="PSUM",
                                   tag="GT")
                GTs = spool.tile([128, CB, 128], dt.bfloat16, tag="GTs")
                if "mm" not in SKIP:
                    for c in range(CB):
                        nc.tensor.matmul(psGT[:, c, :], G[:, c, :], ident_sb[:],
                                         is_transpose=True,
                                         start=(c == 0), stop=(c == CB - 1))
                if "act" not in SKIP:
                    nc.scalar.activation(GTs[:], psGT[:], A.Prelu, alpha=1.0)
                # u in PSUM: per-tile fd row-select + fs identity accumulate
                pu = pupool.tile([128, CB, 128], dt.float32, space="PSUM",
                                 tag="u")
                if "mm" not in SKIP:
                    for c in range(CB):
                        nc.tensor.matmul(out=pu[:, c, :],
                                         lhsT=fdblk_sb[:, blocks[t0 + c], :],
                                         rhs=GTs[:, c, :], start=(c % 4 == 0),
                                         stop=False)
                    for half in range(2):
                        hs = slice(half * 4, half * 4 + 4)
                        nc.tensor.matmul(
                            out=pu[:, hs, :].rearrange("p c d -> p (c d)"),
                            lhsT=ident_sb[:],
                            rhs=fsT[:, 0, E0 + half * 512:E0 + half * 512 + 512],
                            start=False, stop=True)
                uT = upool.tile([128, CB * 128], dt.bfloat16, tag="uT")
                if "act" not in SKIP:
                    nc.scalar.activation(uT[:], pu[:].rearrange("p c d -> p (c d)"),
                                         A.Prelu, alpha=SLOPE)
                # logits on PE, edge-major: out[e, h] = sum_f uT[f, e] A[f, h]
                ps_lgE = pepool.tile([128, CB, 4], dt.float32, space="PSUM",
                                     tag="lgE")
                if "mm" not in SKIP:
                    for c in range(CB):
                        nc.tensor.matmul(out=ps_lgE[:, c, :],
                                         lhsT=uT[:, c * 128:(c + 1) * 128],
                                         rhs=attnA_sb[:], start=(c == 0),
                                         stop=(c == CB - 1))
                # transposes to edge-major
                psT_fs = ptpool.tile([128, CB, 128], dt.bfloat16, space="PSUM",
                                     tag="Tfs")
                if "mm" not in SKIP:
                    for c in range(CB):
                        csl = slice(E0 + c * 128, E0 + (c + 1) * 128)
                        nc.tensor.matmul(psT_fs[:, c, :], fsT[:, 0, csl],
                                         ident_sb[:], is_transpose=True,
                                         start=(c == 0), stop=(c == CB - 1))
                msg = mpool.tile([128, CB, 136], dt.bfloat16, tag="msg")
                if "act" not in SKIP:
                    exdup = msg[:, :, 128:136].rearrange(
                        "p c (h x) -> p c h x", x=2)
                    nc.scalar.activation(
                        exdup,
                        ps_lgE[:].rearrange("p c h -> p c h ()").broadcast_to(
                            (128, CB, 4, 2)), A.Exp)
                if "dve" not in SKIP:
                    m4 = msg[:, :, 0:128].rearrange("p c (h d) -> p c h d", h=4)
                    f4 = psT_fs[:].rearrange("p c (h d) -> p c h d", h=4)
                    e4 = msg[:, :, 128:136].rearrange("p c (h x) -> p c h x", x=2)
                    nc.vector.tensor_tensor(
                        out=m4, in0=f4,
                        in1=e4[:, :, :, 0:1].broadcast_to((128, CB, 4, 32)),
                        op=O.mult)
                for c in range(CB):
                    if "mm" in SKIP or "dve" in SKIP:
                        break
                    ti = t0 + c
                    if tstart[ti]:
                        ps_cur = ppool.tile([128, 136], dt.float32, space="PSUM",
                                            tag="aggps")
                    nc.tensor.matmul(out=ps_cur[:], lhsT=G[:, c, :],
                                     rhs=msg[:, c, :],
                                     start=bool(tstart[ti]), stop=bool(tend[ti]))
                    if tend[ti]:
                        k = blocks[ti]
                        nc.vector.tensor_tensor(
                            out=acc[:, k * 136:(k + 1) * 136],
                            in0=acc[:, k * 136:(k + 1) * 136],
                            in1=ps_cur[:], op=O.add)

        # ---------------- finalize (two half-passes over blocks) ----------------
        accv = acc[:].rearrange("p (b f) -> p b f", f=136)
        den = accv[:, :, 128:136:2]                    # [128, NBLK, 4]
        rd = f1pool.tile([128, NBLK, 4], dt.float32, tag="rd")
        nc.vector.tensor_scalar(out=rd[:], in0=den, scalar1=1e-30, scalar2=None,
                                op0=O.max)
        nc.vector.reciprocal(out=rd[:], in_=rd[:])

        HB = (NBLK + 1) // 2   # 25
        fodv = fo_d[:].rearrange("(p c) d -> p c d", c=NBLK)
        outv = out_d[:].rearrange("(p c) d -> p c d", c=NBLK)
        for kb0 in range(0, NBLK, HB):
            kb1 = min(kb0 + HB, NBLK)
            nb = kb1 - kb0
            fo = fpool.tile([128, HB, 128], dt.float32, tag="fo")
            nc.sync.dma_start(fo[:, :nb, :], fodv[:, kb0:kb1, :])

            s = fpool.tile([128, HB, 128], dt.float32, tag="s")
            s4 = s[:, :nb, :].rearrange("p b (h d) -> p b h d", h=4)
            n4 = accv[:, kb0:kb1, 0:128].rearrange("p b (h d) -> p b h d", h=4)
            r4 = rd[:, kb0:kb1, :].rearrange("p b h -> p b h ()")
            nc.vector.tensor_tensor(out=s4, in0=n4,
                                    in1=r4.broadcast_to((128, nb, 4, 32)),
                                    op=O.mult)
            nc.vector.tensor_tensor(out=s[:, :nb, :], in0=s[:, :nb, :],
                                    in1=fo[:, :nb, :], op=O.add)

            if layer == 0:
                # y = elu(elu(s)) ; elu(x) = relu(x) + exp(min(x,0)) - 1
                sv = s[:, :nb, :]
                m = fo   # residual tile is dead now; reuse as scratch
                mv = m[:, :nb, :]
                nc.vector.tensor_scalar_min(out=mv, in0=sv, scalar1=0.0)
                em = fpool.tile([128, HB, 128], dt.float32, tag="em")
                emv = em[:, :nb, :]
                nc.scalar.activation(emv, mv, A.Exp)
                y1 = fpool.tile([128, HB, 128], dt.float32, tag="y1")
                y1v = y1[:, :nb, :]
                nc.vector.scalar_tensor_tensor(out=y1v, in0=sv, scalar=0.0,
                                               in1=emv, op0=O.max, op1=O.add)
                nc.vector.tensor_scalar_min(out=mv, in0=y1v, scalar1=1.0)
                nc.scalar.activation(emv, mv, A.Exp, bias=negone[:])
                nc.vector.tensor_scalar_max(out=y1v, in0=y1v, scalar1=1.0)
                nc.vector.scalar_tensor_tensor(out=sv, in0=y1v,
                                               scalar=-2.0, in1=emv,
                                               op0=O.add, op1=O.add)
                ost = s
                ow = 128
            else:
                ost = fpool.tile([128, HB, OUT], dt.float32, tag="om")
                nc.vector.tensor_tensor(out=ost[:, :nb, :], in0=s[:, :nb, 0:32],
                                        in1=s[:, :nb, 32:64], op=O.add)
                h23 = fpool.tile([128, HB, OUT], dt.float32, tag="h23")
                nc.vector.tensor_tensor(out=h23[:, :nb, :], in0=s[:, :nb, 64:96],
                                        in1=s[:, :nb, 96:128], op=O.add)
                nc.vector.tensor_tensor(out=ost[:, :nb, :], in0=ost[:, :nb, :],
                                        in1=h23[:, :nb, :], op=O.add)
                nc.vector.tensor_scalar_mul(out=ost[:, :nb, :],
                                            in0=ost[:, :nb, :], scalar1=0.25)
                ow = OUT

            nc.sync.dma_start(outv[:, kb0:kb1, :], ost[:, :nb, :ow])

    nc.compile()
    return nc


# ---------------------------------------------------------------- runner ----
def make_runner(nc, n_cores, use_donate=False):
    install_neuronx_cc_hook()
    partition_name = nc.partition_id_tensor.name if nc.partition_id_tensor else None
    in_names, out_names, out_avals, zero_outs = [], [], [], []
    for alloc in nc.m.functions[0].allocations:
        if not isinstance(alloc, mybir.MemoryLocationSet):
            continue
        name = alloc.memorylocations[0].name
        if alloc.kind == "ExternalInput":
            if name != partition_name:
                in_names.append(name)
        elif alloc.kind == "ExternalOutput":
            dtp = mybir.dt.np(alloc.dtype)
            out_avals.append(jax.core.ShapedArray(tuple(alloc.tensor_shape), dtp))
            out_names.append(name)
            zero_outs.append(np.zeros(tuple(alloc.tensor_shape), dtp))
    n_params = len(in_names)
    n_outs = len(out_names)
    in_names.extend(out_names)
    if partition_name is not None:
        in_names.append(partition_name)
    donate = tuple(range(n_params, n_params + n_outs))

    def _body(*args):
        operands = list(args)
        if partition_name is not None:
            operands.append(bass2jax.partition_id_tensor())
        outs = _bass_exec_p.bind(
            *operands, out_avals=tuple(out_avals), in_names=tuple(in_names),
            out_names=tuple(out_names), lowering_input_output_aliases=(),
            sim_require_finite=True, sim_require_nnan=True, nc=nc)
        return tuple(outs)

    devices = jax.devices()[:n_cores]
    mesh = Mesh(np.asarray(devices), ("core",))
    sharded = jax.jit(
        shard_map(_body, mesh=mesh,
                  in_specs=(PartitionSpec("core"),) * (n_params + n_outs),
                  out_specs=(PartitionSpec("core"),) * n_outs,
                  check_rep=False),
        donate_argnums=(donate if use_donate else ()), keep_unused=True)

    class Runner:
        def __init__(self):
            self.in_names = in_names; self.out_names = out_names
            self.real_in_names = in_names[:n_params]
            self.out_avals = out_avals; self.n_cores = n_cores
        def prep(self, in_maps):
            concat = [np.concatenate([m[nm] for m in in_maps], axis=0) for nm in self.real_in_names]
            concat += [np.concatenate([z]*n_cores, axis=0) for z in zero_outs]
            sh = NamedSharding(mesh, PartitionSpec("core"))
            return [jax.device_put(a, sh) for a in concat]
        def run(self, dev_args):
            return sharded(*dev_args)
        def run_np(self, in_maps):
            outs = self.run(self.prep(in_maps))
            return [
                {nm: np.asarray(outs[i]).reshape(n_cores, *out_avals[i].shape)[c]
                 for i, nm in enumerate(out_names)}
                for c in range(n_cores)]
        def time_steady(self, dev_args, iters=6, warmup=2):
            for _ in range(warmup):
                jax.block_until_ready(self.run(dev_args))
            ts = []
            for _ in range(iters):
                t0 = time.perf_counter()
                jax.block_until_ready(self.run(dev_args))
                ts.append(time.perf_counter() - t0)
            return min(ts), ts
    return Runner()


# ------------------------------------------------------------- host glue ----
def make_consts(attn):
    """attnA [128, 4]: block-diagonal attention vectors; ident; iota."""
    attnA = np.zeros((128, 4), f32)
    for h in range(HEADS):
        attnA[h * OUT:(h + 1) * OUT, h] = attn[h]
    ident = np.eye(128, dtype=f32)
    iota = np.tile(np.arange(128, dtype=f32)[None, :], (128, 1))
    return attnA.astype(bf16), ident.astype(bf16), iota.astype(bf16)


def layer_in_maps(plans, featsT_bf, feats_full, W_src, W_dst, b_src, b_dst,
                  attn, bias):
    """Build per-core in_maps for one layer launch (tables projected on host)."""
    attnA, ident, iota = make_consts(np.asarray(attn, f32))
    bc = (np.asarray(b_src, f32) + np.asarray(b_dst, f32)).reshape(1, -1)
    fo_extra = (np.asarray(bias, f32) + np.asarray(b_src, f32)).reshape(1, -1)
    ff = feats_full.astype(f32)
    fs_all = ff @ np.asarray(W_src, f32)              # [N, 128] unbiased
    fd_all = ff @ np.asarray(W_dst, f32) + bc         # [N, 128] biased
    rows_src = (np.arange(N) % 128) * NCH_SRC + np.arange(N) // 128
    tbl_src = np.zeros((NPAD_SRC, 128), bf16)
    tbl_src[rows_src] = fs_all.astype(bf16)
    rows_dst = (np.arange(OWN) % 128) * NCH_DST + np.arange(OWN) // 128
    in_maps = []
    for c, p in enumerate(plans):
        base = c * OWN
        def wrap16rep(a):
            return np.tile(a.reshape(-1, 16).T, (8, 1)).copy()
        fo = np.zeros((NPAD_DST, 128), f32)
        fo[:OWN] = feats_full[base:base + OWN].astype(f32) + fo_extra
        # permute rows: r = p*NBLK + c holds own node c*128 + p
        fo = np.ascontiguousarray(
            fo.reshape(NBLK, 128, 128).transpose(1, 0, 2).reshape(NPAD_DST, 128))
        tbl_dst = np.zeros((NPAD_DST, 128), bf16)
        tbl_dst[rows_dst] = fd_all[base:base + OWN].astype(bf16)
        in_maps.append(dict(
            tbl_src=tbl_src, tbl_dst=tbl_dst, fo=fo,
            ident=ident, attnA=attnA, iota=iota,
            dstloc2=p["dstloc2"],
            idx_src=wrap16rep(p["idx_src"]),
        ))
    return in_maps


def unpermute_out(o):
    """[NPAD_DST, w] permuted (p-major) -> [OWN, w] natural order."""
    w = o.shape[1]
    return o.reshape(128, NBLK, w).transpose(1, 0, 2).reshape(NPAD_DST, w)[:OWN]


class TwoLayerRunner:
    def __init__(self, src, dst, verbose=False):
        self.plans = make_plans(src, dst)
        self.T = self.plans[0]["T"]
        if verbose:
            print(f"common T={self.T} tiles ({self.T*128} idx slots)")
        self.nc0 = build_layer(self.plans[0], layer=0)
        self.nc1 = build_layer(self.plans[0], layer=1)
        self.r0 = make_runner(self.nc0, N_CORES)
        self.r1 = make_runner(self.nc1, N_CORES)

    def __call__(self, feats, inp):
        featsT_bf = np.ascontiguousarray(feats.T).astype(bf16)
        m0 = layer_in_maps(self.plans, featsT_bf, feats,
                           inp["W_src0"], inp["W_dst0"], inp["b_src0"][:],
                           inp["b_dst0"][:], inp["attn0"], inp["bias0"])
        outs0 = self.r0.run_np(m0)
        h1 = np.concatenate([unpermute_out(o["out_own"]) for o in outs0],
                            axis=0)  # [N, 128]
        h1T_bf = np.ascontiguousarray(h1.T).astype(bf16)
        m1 = layer_in_maps(self.plans, h1T_bf, h1,
                           inp["W_src1"], inp["W_dst1"], inp["b_src1"][:],
                           inp["b_dst1"][:], inp["attn1"], inp["bias1"])
        outs1 = self.r1.run_np(m1)
        out = np.concatenate([unpermute_out(o["out_own"]) for o in outs1],
                             axis=0)  # [N, 32]
        return h1, out


_TLR_CACHE = {}


def kernel(**inputs):
    inputs = {k: np.asarray(v) for k, v in inputs.items()}
    src = inputs["src"].astype(np.int64)
    dst = inputs["dst"].astype(np.int64)
    feats = inputs["feats"].astype(np.float32)
    kh = hash((src.tobytes(), dst.tobytes()))
    if kh not in _TLR_CACHE:
        _TLR_CACHE[kh] = TwoLayerRunner(src, dst)
    tlr = _TLR_CACHE[kh]
    _h1, out = tlr(feats, inputs)
    return out.astype(np.float32)


_NULL_CACHE = {}


def null_baseline():
    """Steady-state wall of a near-empty 8-core launch (dispatch overhead)."""
    if "t" in _NULL_CACHE:
        return _NULL_CACHE["t"]
    nc = bacc.Bacc("TRN2", target_bir_lowering=False, debug=False,
                   num_devices=N_CORES)
    x = nc.dram_tensor("x", (128, 128), dt.float32, kind="ExternalInput")
    y = nc.dram_tensor("y", (128, 128), dt.float32, kind="ExternalOutput")
    with tile.TileContext(nc) as tc, ExitStack() as ctx:
        pool = ctx.enter_context(tc.tile_pool(name="sbuf", bufs=2))
        t = pool.tile([128, 128], dt.float32)
        nc.sync.dma_start(t[:], x[:])
        nc.sync.dma_start(y[:], t[:])
    nc.compile()
    r = make_runner(nc, N_CORES)
    xs = np.zeros((128, 128), np.float32)
    dev = r.prep([{"x": xs}] * N_CORES)
    best, _ = r.time_steady(dev, iters=8, warmup=2)
    _NULL_CACHE["t"] = best
    return best


# revision 29
# speedup vs baseline: 2.4981x; 2.4981x over previous
"""Trainium2 Bass kernel for nn_GATv2_23278722744604.

2-layer GATv2 (N=50000 nodes, E=800000 edges, 128 feats, 4 heads x 32).
Sharding: destination-node blocks across 8 NeuronCores; edges routed to the
owner of their dst node.

v2 design (per layer, per core):
- project features into DRAM tables (row-permuted so table writes are
  contiguous per partition);
- per 4096-edge chunk, dma_gather with transpose=True fetches projected
  src/dst rows in FEATURE-major layout [128f, E];
- u = fsT + fdT (DVE), lr = prelu(u) (Act), logits = attn-block matmul on PE
  (contraction over the feature partition dim), ex = exp (Act);
- PE transposes bring fs and ex back to edge-major; messages m = ex * fs;
- one-hot G built by is_equal (iota vs dup-pair dstloc), scatter-add via
  G^T @ [m | exdup] matmuls accumulated in PSUM per dst block;
- finalize: out = num/den + (residual + biases), layer0 double-elu /
  layer1 head-mean.
"""
import os
import time
import numpy as np
import ml_dtypes
import jax
from jax.sharding import Mesh, PartitionSpec, NamedSharding
from jax.experimental.shard_map import shard_map
import concourse.bass as bass
import concourse.bacc as bacc
import concourse.mybir as mybir
import concourse.tile as tile
from concourse import library_config, bass2jax
from concourse.bass2jax import _bass_exec_p, install_neuronx_cc_hook
from contextlib import ExitStack

bf16 = ml_dtypes.bfloat16
f32 = np.float32
dt = mybir.dt
A = mybir.ActivationFunctionType
O = mybir.AluOpType
SKIP = set()

N = 50000
D = 128
HEADS = 4
OUT = 32
N_CORES = 8
OWN = N // N_CORES            # 6250
NBLK = (OWN + 127) // 128     # 49
NCH_SRC = (N + 127) // 128    # 391
NPAD_SRC = 128 * NCH_SRC      # 50048
NCH_DST = NBLK                # 49
NPAD_DST = 128 * NCH_DST      # 6272
BUCKET = 32768
GCALL = 8192                  # idxs per dma_gather call
TPC = GCALL // 128            # 32 tiles per chunk
CB = 8                        # tiles per batch
SLOPE = 0.2


def srcrow(n):
    """Permuted table row for global node n (partition-major layout)."""
    return (n % 128) * NCH_SRC + n // 128


def dstrow(dl):
    return (dl % 128) * NCH_DST + dl // 128


FLEX0 = NPAD_SRC - BUCKET     # 17280: bucket1 table base (buckets overlap)


def plan_core_groups(src, dst, core):
    """Per-block edge lists for one core (permuted src rows, sorted)."""
    base = core * OWN
    sel = (dst >= base) & (dst < base + OWN)
    es = src[sel].astype(np.int64)
    ed = (dst[sel] - base).astype(np.int64)
    rs = (es % 128) * NCH_SRC + es // 128
    blks = {}
    for k in range(NBLK):
        m = ed // 128 == k
        r, sl = rs[m], ed[m] % 128
        o = np.argsort(r, kind="stable")   # rows ascending: must0,flex,must1
        blks[k] = (r[o], sl[o])
    return blks


def make_plans(src, dst):
    """Common-structure plans for all cores (same NEFF across cores).

    src-row buckets overlap: bucket0 = rows [0, BUCKET), bucket1 = rows
    [FLEX0, NPAD_SRC). Rows in [FLEX0, BUCKET) may go to either bucket, which
    lets every core fill its bucket0 group for block k with exactly
    128*m[k] edges (zero padding there).
    """
    all_blks = [plan_core_groups(src, dst, c) for c in range(N_CORES)]
    ntiles = {}
    for k in range(NBLK):
        elig0 = [int(np.searchsorted(b[k][0], BUCKET)) for b in all_blks]
        must0 = [int(np.searchsorted(b[k][0], FLEX0)) for b in all_blks]
        m_k = min(e // 128 for e in elig0)
        assert m_k * 128 >= max(must0), (k, m_k, max(must0))
        n_k = max((len(b[k][0]) - m_k * 128 + 127) // 128 for b in all_blks)
        ntiles[(0, k)] = m_k
        ntiles[(1, k)] = n_k
    m0fill = {k: ntiles[(0, k)] for k in range(NBLK)}
    run_lens = [sum(ntiles[(b, k)] for k in range(NBLK)) for b in range(2)]
    for b in range(2):
        pad = (-run_lens[b]) % TPC
        ntiles[(b, NBLK - 1)] += pad
        run_lens[b] += pad
    blocks, start, end = [], [], []
    for b in range(2):
        for k in range(NBLK):
            n = ntiles[(b, k)]
            if n == 0:
                continue
            blocks += [k] * n
            start += [True] + [False] * (n - 1)
            end += [False] * (n - 1) + [True]
    T = len(blocks)
    n_run0 = run_lens[0]
    common = dict(T=T, n_run0=n_run0, blocks=blocks,
                  start=np.array(start), end=np.array(end))

    plans = []
    for c in range(N_CORES):
        blks = all_blks[c]
        idx_src, dstloc_cols = [], []
        for b in range(2):
            for k in range(NBLK):
                n = ntiles[(b, k)]
                if n == 0:
                    continue
                rows, slots = blks[k]
                s0 = m0fill[k] * 128                 # bucket0 edge count
                if b == 0:
                    e_s, e_sl = rows[:s0], slots[:s0]
                else:
                    e_s, e_sl = rows[s0:] - FLEX0, slots[s0:]
                cap = n * 128
                pad = cap - len(e_s)
                assert pad >= 0, (b, k, n, len(e_s))
                e_s = np.concatenate([e_s, np.zeros(pad, np.int64)])
                e_sl = np.concatenate([e_sl, np.full(pad, 255, np.int64)])
                idx_src.append(e_s)
                dstloc_cols.append(e_sl.reshape(n, 128))
        p = dict(common)
        p["idx_src"] = np.concatenate(idx_src).astype(np.int16)
        dl = np.concatenate(dstloc_cols, axis=0)          # [T, 128] slots
        # transposed + duplicated pairs: [128, T, 2] -> [128, 2T]
        dlT = dl.T.astype(np.float32).astype(bf16)         # [128, T]
        p["dstloc2"] = np.repeat(dlT, 2, axis=1)           # [128, 2T]
        p["own_base"] = c * OWN
        plans.append(p)
    return plans


def build_layer(plan, layer):
    """Build the per-core NEFF for one GATv2 layer given the edge plan."""
    T = plan["T"]
    blocks, tstart, tend = plan["blocks"], plan["start"], plan["end"]
    n_run0 = plan["n_run0"]
    NIDX = T * 128
    NCHUNK = NIDX // GCALL
    chunks0 = n_run0 * 128 // GCALL   # chunks in bucket0 run

    nc = bacc.Bacc("TRN2", target_bir_lowering=False, debug=False,
                   num_devices=N_CORES)
    # projected tables are computed on host; rows permuted r = (n%128)*NCH + n//128
    tbl_src_d = nc.dram_tensor("tbl_src", (NPAD_SRC, 128), dt.bfloat16, kind="ExternalInput")
    tbl_dst_d = nc.dram_tensor("tbl_dst", (NPAD_DST, 128), dt.bfloat16, kind="ExternalInput")
    # residual+bias input and output, both in permuted row order r = p*NBLK + c
    fo_d = nc.dram_tensor("fo", (NPAD_DST, 128), dt.float32, kind="ExternalInput")
    ident_d = nc.dram_tensor("ident", (128, 128), dt.bfloat16, kind="ExternalInput")
    attnA_d = nc.dram_tensor("attnA", (128, 4), dt.bfloat16, kind="ExternalInput")
    iota_d = nc.dram_tensor("iota", (128, 128), dt.bfloat16, kind="ExternalInput")
    dstloc2_d = nc.dram_tensor("dstloc2", (128, 2 * T), dt.bfloat16, kind="ExternalInput")
    idx_src_d = nc.dram_tensor("idx_src", (128, NIDX // 16), dt.int16, kind="ExternalInput")
    OUTW = 128 if layer == 0 else OUT
    out_d = nc.dram_tensor("out_own", (NPAD_DST, OUTW), dt.float32, kind="ExternalOutput")

    with tile.TileContext(nc) as tc, ExitStack() as ctx:
        cpool = ctx.enter_context(tc.tile_pool(name="const", bufs=1))
        ipool = ctx.enter_context(tc.tile_pool(name="idx", bufs=2))
        gpool = ctx.enter_context(tc.tile_pool(name="gath", bufs=2))
        upool = ctx.enter_context(tc.tile_pool(name="u", bufs=2))
        mpool = ctx.enter_context(tc.tile_pool(name="msg", bufs=3))
        spool = ctx.enter_context(tc.tile_pool(name="scratch", bufs=3))
        apool = ctx.enter_context(tc.tile_pool(name="acc", bufs=1))
        ppool = ctx.enter_context(tc.tile_pool(name="psagg", bufs=2, space="PSUM"))
        ptpool = ctx.enter_context(tc.tile_pool(name="psT", bufs=1, space="PSUM"))
        pgpool = ctx.enter_context(tc.tile_pool(name="psGT", bufs=2, space="PSUM"))
        pupool = ctx.enter_context(tc.tile_pool(name="psu", bufs=1, space="PSUM"))
        pepool = ctx.enter_context(tc.tile_pool(name="psex", bufs=1, space="PSUM"))
        fpool = ctx.enter_context(tc.tile_pool(name="fin", bufs=1))
        f1pool = ctx.enter_context(tc.tile_pool(name="fin1", bufs=1))

        nc.gpsimd.load_library(library_config.mlp)

        # ---------------- constants ----------------
        ident_sb = cpool.tile([128, 128], dt.bfloat16)
        attnA_sb = cpool.tile([128, 4], dt.bfloat16)
        iota_sb = cpool.tile([128, 128], dt.bfloat16)
        dstloc_sb = cpool.tile([128, 2 * T], dt.bfloat16)
        nc.sync.dma_start(ident_sb[:], ident_d[:])
        nc.sync.dma_start(attnA_sb[:], attnA_d[:])
        nc.sync.dma_start(iota_sb[:], iota_d[:])
        nc.sync.dma_start(dstloc_sb[:], dstloc2_d[:])
        fdblk_sb = cpool.tile([128, NBLK, 128], dt.bfloat16)
        nc.sync.dma_start(fdblk_sb[:],
                          tbl_dst_d[:].rearrange("(p c) d -> p c d", c=NBLK))


        # ---------------- edge phase ----------------
        acc = apool.tile([128, NBLK * 136], dt.float32)
        nc.vector.memset(acc[:], 0.0)
        negone = cpool.tile([128, 1], dt.float32)
        nc.vector.memset(negone[:], -1.0)

        ps_cur = None
        IW = GCALL // 16
        for ch in range(NCHUNK):
            fsT = gpool.tile([128, 1, GCALL], dt.bfloat16, tag="fsT")
            isrc = ipool.tile([128, IW], dt.int16, tag="isrc")
            nc.sync.dma_start(isrc[:], idx_src_d[:, ch * IW:(ch + 1) * IW])
            if ch < chunks0:
                src_tab = tbl_src_d[0:BUCKET, :]
            else:
                src_tab = tbl_src_d[FLEX0:NPAD_SRC, :]
            if "gather" not in SKIP:
                nc.gpsimd.dma_gather(
                    out_ap=fsT[:], in_ap=src_tab,
                    idxs_ap=isrc[:],
                    num_idxs=GCALL, num_idxs_reg=GCALL, elem_size=128,
                    transpose=True, single_packet=False)
            else:
                nc.vector.memset(fsT[:], 0.5)

            for sb in range(TPC // CB):
                t0 = ch * TPC + sb * CB
                E0 = sb * CB * 128
                esl = slice(E0, E0 + CB * 128)
                # one-hot G for this batch (also used for fd selection)
                G = spool.tile([128, CB, 128], dt.bfloat16, tag="G")
                if "dve" not in SKIP:
                    g4 = G[:].rearrange("p c (m x) -> p c m x", x=2)
                    io2 = iota_sb[:].rearrange("p (m x) -> p () m x", x=2)
                    dl2 = dstloc_sb[:, 2 * t0:2 * (t0 + CB)].rearrange(
                        "p (c x) -> p c () x", x=2)
                    nc.vector.tensor_tensor(
                        out=g4, in0=io2.broadcast_to((128, CB, 64, 2)),
                        in1=dl2.broadcast_to((128, CB, 64, 2)), op=O.is_equal)
                # GT = transpose(G) -> SBUF (matmul rhs)
                psGT = pgpool.tile([128, CB, 128], dt.bfloat16, space="PSUM",
                                   tag="GT")
                GTs = spool.tile([128, CB, 128], dt.bfloat16, tag="GTs")
                if "mm" not in SKIP:
                    for c in range(CB):
                        nc.tensor.matmul(psGT[:, c, :], G[:, c, :], ident_sb[:],
                                         is_transpose=True,
                                         start=(c == 0), stop=(c == CB - 1))
                if "act" not in SKIP:
                    nc.scalar.activation(GTs[:], psGT[:], A.Prelu, alpha=1.0)
                # u in PSUM: per-tile fd row-select + fs identity accumulate
                pu = pupool.tile([128, CB, 128], dt.float32, space="PSUM",
                                 tag="u")
                if "mm" not in SKIP:
                    for c in range(CB):
                        nc.tensor.matmul(out=pu[:, c, :],
                                         lhsT=fdblk_sb[:, blocks[t0 + c], :],
                                         rhs=GTs[:, c, :], start=(c % 4 == 0),
                                         stop=False)
                    for half in range(2):
                        hs = slice(half * 4, half * 4 + 4)
                        nc.tensor.matmul(
                            out=pu[:, hs, :].rearrange("p c d -> p (c d)"),
                            lhsT=ident_sb[:],
                            rhs=fsT[:, 0, E0 + half * 512:E0 + half * 512 + 512],
                            start=False, stop=True)
                uT = upool.tile([128, CB * 128], dt.bfloat16, tag="uT")
                if "act" not in SKIP:
                    nc.scalar.activation(uT[:], pu[:].rearrange("p c d -> p (c d)"),
                                         A.Prelu, alpha=SLOPE)
                # logits on PE, edge-major: out[e, h] = sum_f uT[f, e] A[f, h]
                ps_lgE = pepool.tile([128, CB, 4], dt.float32, space="PSUM",
                                     tag="lgE")
                if "mm" not in SKIP:
                    for c in range(CB):
                        nc.tensor.matmul(out=ps_lgE[:, c, :],
                                         lhsT=uT[:, c * 128:(c + 1) * 128],
                                         rhs=attnA_sb[:], start=(c == 0),
                                         stop=(c == CB - 1))
                # transposes to edge-major
                psT_fs = ptpool.tile([128, CB, 128], dt.bfloat16, space="PSUM",
                                     tag="Tfs")
                if "mm" not in SKIP:
                    for c in range(CB):
                        csl = slice(E0 + c * 128, E0 + (c + 1) * 128)
                        nc.tensor.matmul(psT_fs[:, c, :], fsT[:, 0, csl],
                                         ident_sb[:], is_transpose=True,
                                         start=(c == 0), stop=(c == CB - 1))
                msg = mpool.tile([128, CB, 136], dt.bfloat16, tag="msg")
                if "act" not in SKIP:
                    exdup = msg[:, :, 128:136].rearrange(
                        "p c (h x) -> p c h x", x=2)
                    nc.scalar.activation(
                        exdup,
                        ps_lgE[:].rearrange("p c h -> p c h ()").broadcast_to(
                            (128, CB, 4, 2)), A.Exp)
                if "dve" not in SKIP:
                    m4 = msg[:, :, 0:128].rearrange("p c (h d) -> p c h d", h=4)
                    f4 = psT_fs[:].rearrange("p c (h d) -> p c h d", h=4)
                    e4 = msg[:, :, 128:136].rearrange("p c (h x) -> p c h x", x=2)
                    nc.vector.tensor_tensor(
                        out=m4, in0=f4,
                        in1=e4[:, :, :, 0:1].broadcast_to((128, CB, 4, 32)),
                        op=O.mult)
                for c in range(CB):
                    if "mm" in SKIP or "dve" in SKIP:
                        break
                    ti = t0 + c
                    if tstart[ti]:
                        ps_cur = ppool.tile([128, 136], dt.float32, space="PSUM",
                                            tag="aggps")
                    nc.tensor.matmul(out=ps_cur[:], lhsT=G[:, c, :],
                                     rhs=msg[:, c, :],
                                     start=bool(tstart[ti]), stop=bool(tend[ti]))
                    if tend[ti]:
                        k = blocks[ti]
                        nc.vector.tensor_tensor(
                            out=acc[:, k * 136:(k + 1) * 136],
                            in0=acc[:, k * 136:(k + 1) * 136],
                            in1=ps_cur[:], op=O.add)

        # ---------------- finalize (two half-passes over blocks) ----------------
        accv = acc[:].rearrange("p (b f) -> p b f", f=136)
        den = accv[:, :, 128:136:2]                    # [128, NBLK, 4]
        rd = f1pool.tile([128, NBLK, 4], dt.float32, tag="rd")
        nc.vector.tensor_scalar(out=rd[:], in0=den, scalar1=1e-30, scalar2=None,
                                op0=O.max)
        nc.vector.reciprocal(out=rd[:], in_=rd[:])

        HB = (NBLK + 1) // 2   # 25
        fodv = fo_d[:].rearrange("(p c) d -> p c d", c=NBLK)
        outv = out_d[:].rearrange("(p c) d -> p c d", c=NBLK)
        for kb0 in range(0, NBLK, HB):
            kb1 = min(kb0 + HB, NBLK)
            nb = kb1 - kb0
            fo = fpool.tile([128, HB, 128], dt.float32, tag="fo")
            nc.sync.dma_start(fo[:, :nb, :], fodv[:, kb0:kb1, :])

            s = fpool.tile([128, HB, 128], dt.float32, tag="s")
            s4 = s[:, :nb, :].rearrange("p b (h d) -> p b h d", h=4)
            n4 = accv[:, kb0:kb1, 0:128].rearrange("p b (h d) -> p b h d", h=4)
            r4 = rd[:, kb0:kb1, :].rearrange("p b h -> p b h ()")
            nc.vector.tensor_tensor(out=s4, in0=n4,
                                    in1=r4.broadcast_to((128, nb, 4, 32)),
                                    op=O.mult)
            nc.vector.tensor_tensor(out=s[:, :nb, :], in0=s[:, :nb, :],
                                    in1=fo[:, :nb, :], op=O.add)

            if layer == 0:
                # y = elu(elu(s)) ; elu(x) = relu(x) + exp(min(x,0)) - 1
                sv = s[:, :nb, :]
                m = fo   # residual tile is dead now; reuse as scratch
                mv = m[:, :nb, :]
                nc.vector.tensor_scalar_min(out=mv, in0=sv, scalar1=0.0)
                em = fpool.tile([128, HB, 128], dt.float32, tag="em")
                emv = em[:, :nb, :]
                nc.scalar.activation(emv, mv, A.Exp)
                y1 = fpool.tile([128, HB, 128], dt.float32, tag="y1")
                y1v = y1[:, :nb, :]
                nc.vector.scalar_tensor_tensor(out=y1v, in0=sv, scalar=0.0,
                                               in1=emv, op0=O.max, op1=O.add)
                nc.vector.tensor_scalar_min(out=mv, in0=y1v, scalar1=1.0)
                nc.scalar.activation(emv, mv, A.Exp, bias=negone[:])
                nc.vector.tensor_scalar_max(out=y1v, in0=y1v, scalar1=1.0)
                nc.vector.scalar_tensor_tensor(out=sv, in0=y1v,
                                               scalar=-2.0, in1=emv,
                                               op0=O.add, op1=O.add)
                ost = s
                ow = 128
            else:
                ost = fpool.tile([128, HB, OUT], dt.float32, tag="om")
                nc.vector.tensor_tensor(out=ost[:, :nb, :], in0=s[:, :nb, 0:32],
                                        in1=s[:, :nb, 32:64], op=O.add)
                h23 = fpool.tile([128, HB, OUT], dt.float32, tag="h23")
                nc.vector.tensor_tensor(out=h23[:, :nb, :], in0=s[:, :nb, 64:96],
                                        in1=s[:, :nb, 96:128], op=O.add)
                nc.vector.tensor_tensor(out=ost[:, :nb, :], in0=ost[:, :nb, :],
                                        in1=h23[:, :nb, :], op=O.add)
                nc.vector.tensor_scalar_mul(out=ost[:, :nb, :],
                                            in0=ost[:, :nb, :], scalar1=0.25)
                ow = OUT

            nc.sync.dma_start(outv[:, kb0:kb1, :], ost[:, :nb, :ow])

    nc.compile()
    return nc


# ---------------------------------------------------------------- runner ----
def make_runner(nc, n_cores, use_donate=False):
    install_neuronx_cc_hook()
    partition_name = nc.partition_id_tensor.name if nc.partition_id_tensor else None
    in_names, out_names, out_avals, zero_outs = [], [], [], []
    for alloc in nc.m.functions[0].allocations:
        if not isinstance(alloc, mybir.MemoryLocationSet):
            continue
        name = alloc.memorylocations[0].name
        if alloc.kind == "ExternalInput":
            if name != partition_name:
                in_names.append(name)
        elif alloc.kind == "ExternalOutput":
            dtp = mybir.dt.np(alloc.dtype)
            out_avals.append(jax.core.ShapedArray(tuple(alloc.tensor_shape), dtp))
            out_names.append(name)
            zero_outs.append(np.zeros(tuple(alloc.tensor_shape), dtp))
    n_params = len(in_names)
    n_outs = len(out_names)
    in_names.extend(out_names)
    if partition_name is not None:
        in_names.append(partition_name)
    donate = tuple(range(n_params, n_params + n_outs))

    def _body(*args):
        operands = list(args)
        if partition_name is not None:
            operands.append(bass2jax.partition_id_tensor())
        outs = _bass_exec_p.bind(
            *operands, out_avals=tuple(out_avals), in_names=tuple(in_names),
            out_names=tuple(out_names), lowering_input_output_aliases=(),
            sim_require_finite=True, sim_require_nnan=True, nc=nc)
        return tuple(outs)

    devices = jax.devices()[:n_cores]
    mesh = Mesh(np.asarray(devices), ("core",))
    sharded = jax.jit(
        shard_map(_body, mesh=mesh,
                  in_specs=(PartitionSpec("core"),) * (n_params + n_outs),
                  out_specs=(PartitionSpec("core"),) * n_outs,
                  check_rep=False),
        donate_argnums=(donate if use_donate else ()), keep_unused=True)

    class Runner:
        def __init__(self):
            self.in_names = in_names; self.out_names = out_names
            self.real_in_names = in_names[:n_params]
            self.out_avals = out_avals; self.n_cores = n_cores
        def prep(self, in_maps):
            concat = [np.concatenate([m[nm] for m in in_maps], axis=0) for nm in self.real_in_names]
            concat += [np.concatenate([z]*n_cores, axis=0) for z in zero_outs]
            sh = NamedSharding(mesh, PartitionSpec("core"))
            return [jax.device_put(a, sh) for a in concat]
        def run(self, dev_args):
            return sharded(*dev_args)
        def run_np(self, in_maps):
            outs = self.run(self.prep(in_maps))
            return [
                {nm: np.asarray(outs[i]).reshape(n_cores, *out_avals[i].shape)[c]
                 for i, nm in enumerate(out_names)}
                for c in range(n_cores)]
        def time_steady(self, dev_args, iters=6, warmup=2):
            for _ in range(warmup):
                jax.block_until_ready(self.run(dev_args))
            ts = []
            for _ in range(iters):
                t0 = time.perf_counter()
                jax.block_until_ready(self.run(dev_args))
                ts.append(time.perf_counter() - t0)
            return min(ts), ts
    return Runner()


# ------------------------------------------------------------- host glue ----
def make_consts(attn):
    """attnA [128, 4]: block-diagonal attention vectors; ident; iota."""
    attnA = np.zeros((128, 4), f32)
    for h in range(HEADS):
        attnA[h * OUT:(h + 1) * OUT, h] = attn[h]
    ident = np.eye(128, dtype=f32)
    iota = np.tile(np.arange(128, dtype=f32)[None, :], (128, 1))
    return attnA.astype(bf16), ident.astype(bf16), iota.astype(bf16)


def layer_in_maps(plans, featsT_bf, feats_full, W_src, W_dst, b_src, b_dst,
                  attn, bias):
    """Build per-core in_maps for one layer launch (tables projected on host)."""
    attnA, ident, iota = make_consts(np.asarray(attn, f32))
    bc = (np.asarray(b_src, f32) + np.asarray(b_dst, f32)).reshape(1, -1)
    fo_extra = (np.asarray(bias, f32) + np.asarray(b_src, f32)).reshape(1, -1)
    ff = feats_full.astype(f32)
    fs_all = ff @ np.asarray(W_src, f32)              # [N, 128] unbiased
    fd_all = ff @ np.asarray(W_dst, f32) + bc         # [N, 128] biased
    rows_src = (np.arange(N) % 128) * NCH_SRC + np.arange(N) // 128
    tbl_src = np.zeros((NPAD_SRC, 128), bf16)
    tbl_src[rows_src] = fs_all.astype(bf16)
    rows_dst = (np.arange(OWN) % 128) * NCH_DST + np.arange(OWN) // 128
    in_maps = []
    for c, p in enumerate(plans):
        base = c * OWN
        def wrap16rep(a):
            return np.tile(a.reshape(-1, 16).T, (8, 1)).copy()
        fo = np.zeros((NPAD_DST, 128), f32)
        fo[:OWN] = feats_full[base:base + OWN].astype(f32) + fo_extra
        # permute rows: r = p*NBLK + c holds own node c*128 + p
        fo = np.ascontiguousarray(
            fo.reshape(NBLK, 128, 128).transpose(1, 0, 2).reshape(NPAD_DST, 128))
        tbl_dst = np.zeros((NPAD_DST, 128), bf16)
        tbl_dst[rows_dst] = fd_all[base:base + OWN].astype(bf16)
        in_maps.append(dict(
            tbl_src=tbl_src, tbl_dst=tbl_dst, fo=fo,
            ident=ident, attnA=attnA, iota=iota,
            dstloc2=p["dstloc2"],
            idx_src=wrap16rep(p["idx_src"]),
        ))
    return in_maps


def unpermute_out(o):
    """[NPAD_DST, w] permuted (p-major) -> [OWN, w] natural order."""
    w = o.shape[1]
    return o.reshape(128, NBLK, w).transpose(1, 0, 2).reshape(NPAD_DST, w)[:OWN]


class TwoLayerRunner:
    def __init__(self, src, dst, verbose=False):
        self.plans = make_plans(src, dst)
        self.T = self.plans[0]["T"]
        if verbose:
            print(f"common T={self.T} tiles ({self.T*128} idx slots)")
        self.nc0 = build_layer(self.plans[0], layer=0)
        self.nc1 = build_layer(self.plans[0], layer=1)
        self.r0 = make_runner(self.nc0, N_CORES)
        self.r1 = make_runner(self.nc1, N_CORES)

    def __call__(self, feats, inp):
        featsT_bf = np.ascontiguousarray(feats.T).astype(bf16)
        m0 = layer_in_maps(self.plans, featsT_bf, feats,
                           inp["W_src0"], inp["W_dst0"], inp["b_src0"][:],
                           inp["b_dst0"][:], inp["attn0"], inp["bias0"])
        outs0 = self.r0.run_np(m0)
        h1 = np.concatenate([unpermute_out(o["out_own"]) for o in outs0],
                            axis=0)  # [N, 128]
        h1T_bf = np.ascontiguousarray(h1.T).astype(bf16)
        m1 = layer_in_maps(self.plans, h1T_bf, h1,
                           inp["W_src1"], inp["W_dst1"], inp["b_src1"][:],
                           inp["b_dst1"][:], inp["attn1"], inp["bias1"])
        outs1 = self.r1.run_np(m1)
        out = np.concatenate([unpermute_out(o["out_own"]) for o in outs1],
                             axis=0)  # [N, 32]
        return h1, out


_TLR_CACHE = {}


def kernel(**inputs):
    inputs = {k: np.asarray(v) for k, v in inputs.items()}
    src = inputs["src"].astype(np.int64)
    dst = inputs["dst"].astype(np.int64)
    feats = inputs["feats"].astype(np.float32)
    kh = hash((src.tobytes(), dst.tobytes()))
    if kh not in _TLR_CACHE:
        _TLR_CACHE[kh] = TwoLayerRunner(src, dst)
    tlr = _TLR_CACHE[kh]
    _h1, out = tlr(feats, inputs)
    return out.astype(np.float32)


_NULL_CACHE = {}


def null_baseline():
    """Steady-state wall of a near-empty 8-core launch (dispatch overhead)."""
    if "t" in _NULL_CACHE:
        return _NULL_CACHE["t"]
    nc = bacc.Bacc("TRN2", target_bir_lowering=False, debug=False,
                   num_devices=N_CORES)
    x = nc.dram_tensor("x", (128, 128), dt.float32, kind="ExternalInput")
    y = nc.dram_tensor("y", (128, 128), dt.float32, kind="ExternalOutput")
    with tile.TileContext(nc) as tc, ExitStack() as ctx:
        pool = ctx.enter_context(tc.tile_pool(name="sbuf", bufs=2))
        t = pool.tile([128, 128], dt.float32)
        nc.sync.dma_start(t[:], x[:])
        nc.sync.dma_start(y[:], t[:])
    nc.compile()
    r = make_runner(nc, N_CORES)
    xs = np.zeros((128, 128), np.float32)
    dev = r.prep([{"x": xs}] * N_CORES)
    best, _ = r.time_steady(dev, iters=30, warmup=3)
    _NULL_CACHE["t"] = best
    return best


# revision 32
# speedup vs baseline: 35.8618x; 14.3557x over previous
"""Trainium2 Bass kernel for nn_GATv2_23278722744604.

2-layer GATv2 (N=50000 nodes, E=800000 edges, 128 feats, 4 heads x 32).
Sharding: destination-node blocks across 8 NeuronCores; edges routed to the
owner of their dst node.

v2 design (per layer, per core):
- project features into DRAM tables (row-permuted so table writes are
  contiguous per partition);
- per 4096-edge chunk, dma_gather with transpose=True fetches projected
  src/dst rows in FEATURE-major layout [128f, E];
- u = fsT + fdT (DVE), lr = prelu(u) (Act), logits = attn-block matmul on PE
  (contraction over the feature partition dim), ex = exp (Act);
- PE transposes bring fs and ex back to edge-major; messages m = ex * fs;
- one-hot G built by is_equal (iota vs dup-pair dstloc), scatter-add via
  G^T @ [m | exdup] matmuls accumulated in PSUM per dst block;
- finalize: out = num/den + (residual + biases), layer0 double-elu /
  layer1 head-mean.
"""
import os
import time
import numpy as np
import ml_dtypes
import jax
from jax.sharding import Mesh, PartitionSpec, NamedSharding
from jax.experimental.shard_map import shard_map
import concourse.bass as bass
import concourse.bacc as bacc
import concourse.mybir as mybir
import concourse.tile as tile
from concourse import library_config, bass2jax
from concourse.bass2jax import _bass_exec_p, install_neuronx_cc_hook
from contextlib import ExitStack

bf16 = ml_dtypes.bfloat16
f32 = np.float32
dt = mybir.dt
A = mybir.ActivationFunctionType
O = mybir.AluOpType
SKIP = set()

N = 50000
D = 128
HEADS = 4
OUT = 32
N_CORES = 8
OWN = N // N_CORES            # 6250
NBLK = (OWN + 127) // 128     # 49
NCH_SRC = (N + 127) // 128    # 391
NPAD_SRC = 128 * NCH_SRC      # 50048
NCH_DST = NBLK                # 49
NPAD_DST = 128 * NCH_DST      # 6272
BUCKET = 32768
GCALL = 8192                  # idxs per dma_gather call
TPC = GCALL // 128            # 32 tiles per chunk
CB = 8                        # tiles per batch
SLOPE = 0.2


def srcrow(n):
    """Permuted table row for global node n (partition-major layout)."""
    return (n % 128) * NCH_SRC + n // 128


def dstrow(dl):
    return (dl % 128) * NCH_DST + dl // 128


FLEX0 = NPAD_SRC - BUCKET     # 17280: bucket1 table base (buckets overlap)


def plan_core_groups(src, dst, core):
    """Per-block edge lists for one core (permuted src rows, sorted)."""
    base = core * OWN
    sel = (dst >= base) & (dst < base + OWN)
    es = src[sel].astype(np.int64)
    ed = (dst[sel] - base).astype(np.int64)
    rs = (es % 128) * NCH_SRC + es // 128
    blks = {}
    for k in range(NBLK):
        m = ed // 128 == k
        r, sl = rs[m], ed[m] % 128
        o = np.argsort(r, kind="stable")   # rows ascending: must0,flex,must1
        blks[k] = (r[o], sl[o])
    return blks


def make_plans(src, dst):
    """Common-structure plans for all cores (same NEFF across cores).

    src-row buckets overlap: bucket0 = rows [0, BUCKET), bucket1 = rows
    [FLEX0, NPAD_SRC). Rows in [FLEX0, BUCKET) may go to either bucket, which
    lets every core fill its bucket0 group for block k with exactly
    128*m[k] edges (zero padding there).
    """
    all_blks = [plan_core_groups(src, dst, c) for c in range(N_CORES)]
    ntiles = {}
    for k in range(NBLK):
        elig0 = [int(np.searchsorted(b[k][0], BUCKET)) for b in all_blks]
        must0 = [int(np.searchsorted(b[k][0], FLEX0)) for b in all_blks]
        m_k = min(e // 128 for e in elig0)
        assert m_k * 128 >= max(must0), (k, m_k, max(must0))
        n_k = max((len(b[k][0]) - m_k * 128 + 127) // 128 for b in all_blks)
        ntiles[(0, k)] = m_k
        ntiles[(1, k)] = n_k
    m0fill = {k: ntiles[(0, k)] for k in range(NBLK)}
    run_lens = [sum(ntiles[(b, k)] for k in range(NBLK)) for b in range(2)]
    for b in range(2):
        pad = (-run_lens[b]) % TPC
        ntiles[(b, NBLK - 1)] += pad
        run_lens[b] += pad
    blocks, start, end = [], [], []
    for b in range(2):
        for k in range(NBLK):
            n = ntiles[(b, k)]
            if n == 0:
                continue
            blocks += [k] * n
            start += [True] + [False] * (n - 1)
            end += [False] * (n - 1) + [True]
    T = len(blocks)
    n_run0 = run_lens[0]
    common = dict(T=T, n_run0=n_run0, blocks=blocks,
                  start=np.array(start), end=np.array(end))

    plans = []
    for c in range(N_CORES):
        blks = all_blks[c]
        idx_src, dstloc_cols = [], []
        for b in range(2):
            for k in range(NBLK):
                n = ntiles[(b, k)]
                if n == 0:
                    continue
                rows, slots = blks[k]
                s0 = m0fill[k] * 128                 # bucket0 edge count
                if b == 0:
                    e_s, e_sl = rows[:s0], slots[:s0]
                else:
                    e_s, e_sl = rows[s0:] - FLEX0, slots[s0:]
                cap = n * 128
                pad = cap - len(e_s)
                assert pad >= 0, (b, k, n, len(e_s))
                e_s = np.concatenate([e_s, np.zeros(pad, np.int64)])
                e_sl = np.concatenate([e_sl, np.full(pad, 255, np.int64)])
                idx_src.append(e_s)
                dstloc_cols.append(e_sl.reshape(n, 128))
        p = dict(common)
        p["idx_src"] = np.concatenate(idx_src).astype(np.int16)
        dl = np.concatenate(dstloc_cols, axis=0)          # [T, 128] slots
        # transposed + duplicated pairs: [128, T, 2] -> [128, 2T]
        dlT = dl.T.astype(np.float32).astype(bf16)         # [128, T]
        p["dstloc2"] = np.repeat(dlT, 2, axis=1)           # [128, 2T]
        p["own_base"] = c * OWN
        plans.append(p)
    return plans


def build_layer(plan, layer):
    """Build the per-core NEFF for one GATv2 layer given the edge plan."""
    T = plan["T"]
    blocks, tstart, tend = plan["blocks"], plan["start"], plan["end"]
    n_run0 = plan["n_run0"]
    NIDX = T * 128
    NCHUNK = NIDX // GCALL
    chunks0 = n_run0 * 128 // GCALL   # chunks in bucket0 run

    nc = bacc.Bacc("TRN2", target_bir_lowering=False, debug=False,
                   num_devices=N_CORES)
    # projected tables are computed on host; rows permuted r = (n%128)*NCH + n//128
    tbl_src_d = nc.dram_tensor("tbl_src", (NPAD_SRC, 128), dt.bfloat16, kind="ExternalInput")
    tbl_dst_d = nc.dram_tensor("tbl_dst", (NPAD_DST, 128), dt.bfloat16, kind="ExternalInput")
    # residual+bias input and output, both in permuted row order r = p*NBLK + c
    fo_d = nc.dram_tensor("fo", (NPAD_DST, 128), dt.float32, kind="ExternalInput")
    ident_d = nc.dram_tensor("ident", (128, 128), dt.bfloat16, kind="ExternalInput")
    attnA_d = nc.dram_tensor("attnA", (128, 4), dt.bfloat16, kind="ExternalInput")
    iota_d = nc.dram_tensor("iota", (128, 128), dt.bfloat16, kind="ExternalInput")
    dstloc2_d = nc.dram_tensor("dstloc2", (128, 2 * T), dt.bfloat16, kind="ExternalInput")
    idx_src_d = nc.dram_tensor("idx_src", (128, NIDX // 16), dt.int16, kind="ExternalInput")
    OUTW = 128 if layer == 0 else OUT
    out_d = nc.dram_tensor("out_own", (NPAD_DST, OUTW), dt.float32, kind="ExternalOutput")

    with tile.TileContext(nc) as tc, ExitStack() as ctx:
        cpool = ctx.enter_context(tc.tile_pool(name="const", bufs=1))
        ipool = ctx.enter_context(tc.tile_pool(name="idx", bufs=2))
        gpool = ctx.enter_context(tc.tile_pool(name="gath", bufs=2))
        upool = ctx.enter_context(tc.tile_pool(name="u", bufs=2))
        mpool = ctx.enter_context(tc.tile_pool(name="msg", bufs=3))
        spool = ctx.enter_context(tc.tile_pool(name="scratch", bufs=3))
        apool = ctx.enter_context(tc.tile_pool(name="acc", bufs=1))
        ppool = ctx.enter_context(tc.tile_pool(name="psagg", bufs=2, space="PSUM"))
        ptpool = ctx.enter_context(tc.tile_pool(name="psT", bufs=1, space="PSUM"))
        pgpool = ctx.enter_context(tc.tile_pool(name="psGT", bufs=2, space="PSUM"))
        pupool = ctx.enter_context(tc.tile_pool(name="psu", bufs=2, space="PSUM"))
        pepool = ctx.enter_context(tc.tile_pool(name="psex", bufs=1, space="PSUM"))
        fpool = ctx.enter_context(tc.tile_pool(name="fin", bufs=1))
        f1pool = ctx.enter_context(tc.tile_pool(name="fin1", bufs=1))

        nc.gpsimd.load_library(library_config.mlp)

        # ---------------- constants ----------------
        ident_sb = cpool.tile([128, 128], dt.bfloat16)
        attnA_sb = cpool.tile([128, 4], dt.bfloat16)
        iota_sb = cpool.tile([128, 128], dt.bfloat16)
        dstloc_sb = cpool.tile([128, 2 * T], dt.bfloat16)
        nc.sync.dma_start(ident_sb[:], ident_d[:])
        nc.sync.dma_start(attnA_sb[:], attnA_d[:])
        nc.sync.dma_start(iota_sb[:], iota_d[:])
        nc.sync.dma_start(dstloc_sb[:], dstloc2_d[:])
        fdblk_sb = cpool.tile([128, NBLK, 128], dt.bfloat16)
        nc.sync.dma_start(fdblk_sb[:],
                          tbl_dst_d[:].rearrange("(p c) d -> p c d", c=NBLK))


        # ---------------- edge phase ----------------
        acc = apool.tile([128, NBLK * 136], dt.float32)
        nc.vector.memset(acc[:], 0.0)
        negone = cpool.tile([128, 1], dt.float32)
        nc.vector.memset(negone[:], -1.0)

        ps_cur = None
        IW = GCALL // 16
        for ch in range(NCHUNK):
            fsT = gpool.tile([128, 1, GCALL], dt.bfloat16, tag="fsT")
            isrc = ipool.tile([128, IW], dt.int16, tag="isrc")
            nc.sync.dma_start(isrc[:], idx_src_d[:, ch * IW:(ch + 1) * IW])
            if ch < chunks0:
                src_tab = tbl_src_d[0:BUCKET, :]
            else:
                src_tab = tbl_src_d[FLEX0:NPAD_SRC, :]
            if "gather" not in SKIP:
                nc.gpsimd.dma_gather(
                    out_ap=fsT[:], in_ap=src_tab,
                    idxs_ap=isrc[:],
                    num_idxs=GCALL, num_idxs_reg=GCALL, elem_size=128,
                    transpose=True, single_packet=False)
            else:
                nc.vector.memset(fsT[:], 0.5)

            for sb in range(TPC // CB):
                t0 = ch * TPC + sb * CB
                E0 = sb * CB * 128
                esl = slice(E0, E0 + CB * 128)
                # one-hot G for this batch (also used for fd selection)
                G = spool.tile([128, CB, 128], dt.bfloat16, tag="G")
                if "dve" not in SKIP:
                    g4 = G[:].rearrange("p c (m x) -> p c m x", x=2)
                    io2 = iota_sb[:].rearrange("p (m x) -> p () m x", x=2)
                    dl2 = dstloc_sb[:, 2 * t0:2 * (t0 + CB)].rearrange(
                        "p (c x) -> p c () x", x=2)
                    nc.vector.tensor_tensor(
                        out=g4, in0=io2.broadcast_to((128, CB, 64, 2)),
                        in1=dl2.broadcast_to((128, CB, 64, 2)), op=O.is_equal)
                # GT = transpose(G) -> SBUF (matmul rhs)
                psGT = pgpool.tile([128, CB, 128], dt.bfloat16, space="PSUM",
                                   tag="GT")
                GTs = spool.tile([128, CB, 128], dt.bfloat16, tag="GTs")
                if "mm" not in SKIP:
                    for c in range(CB):
                        nc.tensor.matmul(psGT[:, c, :], G[:, c, :], ident_sb[:],
                                         is_transpose=True,
                                         start=(c == 0), stop=(c == CB - 1))
                if "act" not in SKIP:
                    nc.scalar.activation(GTs[:], psGT[:], A.Prelu, alpha=1.0)
                # u in PSUM (half-batch tiles, double-buffered):
                # per-tile fd row-select + fs identity accumulate
                uT = upool.tile([128, CB * 128], dt.bfloat16, tag="uT")
                for half in range(2):
                    pu = pupool.tile([128, 4, 128], dt.float32, space="PSUM",
                                     tag="u")
                    if "mm" not in SKIP:
                        for cc in range(4):
                            c = half * 4 + cc
                            nc.tensor.matmul(out=pu[:, cc, :],
                                             lhsT=fdblk_sb[:, blocks[t0 + c], :],
                                             rhs=GTs[:, c, :], start=(cc == 0),
                                             stop=False)
                        nc.tensor.matmul(
                            out=pu[:].rearrange("p c d -> p (c d)"),
                            lhsT=ident_sb[:],
                            rhs=fsT[:, 0, E0 + half * 512:E0 + half * 512 + 512],
                            start=False, stop=True)
                    if "act" not in SKIP:
                        nc.scalar.activation(
                            uT[:, half * 512:(half + 1) * 512],
                            pu[:].rearrange("p c d -> p (c d)"),
                            A.Prelu, alpha=SLOPE)
                # logits on PE, edge-major: out[e, h] = sum_f uT[f, e] A[f, h]
                ps_lgE = pepool.tile([128, CB, 4], dt.float32, space="PSUM",
                                     tag="lgE")
                if "mm" not in SKIP:
                    for c in range(CB):
                        nc.tensor.matmul(out=ps_lgE[:, c, :],
                                         lhsT=uT[:, c * 128:(c + 1) * 128],
                                         rhs=attnA_sb[:], start=(c == 0),
                                         stop=(c == CB - 1))
                # transposes to edge-major
                psT_fs = ptpool.tile([128, CB, 128], dt.bfloat16, space="PSUM",
                                     tag="Tfs")
                if "mm" not in SKIP:
                    for c in range(CB):
                        csl = slice(E0 + c * 128, E0 + (c + 1) * 128)
                        nc.tensor.matmul(psT_fs[:, c, :], fsT[:, 0, csl],
                                         ident_sb[:], is_transpose=True,
                                         start=(c == 0), stop=(c == CB - 1))
                msg = mpool.tile([128, CB, 136], dt.bfloat16, tag="msg")
                if "act" not in SKIP:
                    exdup = msg[:, :, 128:136].rearrange(
                        "p c (h x) -> p c h x", x=2)
                    nc.scalar.activation(
                        exdup,
                        ps_lgE[:].rearrange("p c h -> p c h ()").broadcast_to(
                            (128, CB, 4, 2)), A.Exp)
                if "dve" not in SKIP:
                    m4 = msg[:, :, 0:128].rearrange("p c (h d) -> p c h d", h=4)
                    f4 = psT_fs[:].rearrange("p c (h d) -> p c h d", h=4)
                    e4 = msg[:, :, 128:136].rearrange("p c (h x) -> p c h x", x=2)
                    nc.vector.tensor_tensor(
                        out=m4, in0=f4,
                        in1=e4[:, :, :, 0:1].broadcast_to((128, CB, 4, 32)),
                        op=O.mult)
                for c in range(CB):
                    if "mm" in SKIP or "dve" in SKIP:
                        break
                    ti = t0 + c
                    if tstart[ti]:
                        ps_cur = ppool.tile([128, 136], dt.float32, space="PSUM",
                                            tag="aggps")
                    nc.tensor.matmul(out=ps_cur[:], lhsT=G[:, c, :],
                                     rhs=msg[:, c, :],
                                     start=bool(tstart[ti]), stop=bool(tend[ti]))
                    if tend[ti]:
                        k = blocks[ti]
                        nc.vector.tensor_tensor(
                            out=acc[:, k * 136:(k + 1) * 136],
                            in0=acc[:, k * 136:(k + 1) * 136],
                            in1=ps_cur[:], op=O.add)

        # ---------------- finalize (two half-passes over blocks) ----------------
        accv = acc[:].rearrange("p (b f) -> p b f", f=136)
        den = accv[:, :, 128:136:2]                    # [128, NBLK, 4]
        rd = f1pool.tile([128, NBLK, 4], dt.float32, tag="rd")
        nc.vector.tensor_scalar(out=rd[:], in0=den, scalar1=1e-30, scalar2=None,
                                op0=O.max)
        nc.vector.reciprocal(out=rd[:], in_=rd[:])

        HB = (NBLK + 1) // 2   # 25
        fodv = fo_d[:].rearrange("(p c) d -> p c d", c=NBLK)
        outv = out_d[:].rearrange("(p c) d -> p c d", c=NBLK)
        for kb0 in range(0, NBLK, HB):
            kb1 = min(kb0 + HB, NBLK)
            nb = kb1 - kb0
            fo = fpool.tile([128, HB, 128], dt.float32, tag="fo")
            nc.sync.dma_start(fo[:, :nb, :], fodv[:, kb0:kb1, :])

            s = fpool.tile([128, HB, 128], dt.float32, tag="s")
            s4 = s[:, :nb, :].rearrange("p b (h d) -> p b h d", h=4)
            n4 = accv[:, kb0:kb1, 0:128].rearrange("p b (h d) -> p b h d", h=4)
            r4 = rd[:, kb0:kb1, :].rearrange("p b h -> p b h ()")
            nc.vector.tensor_tensor(out=s4, in0=n4,
                                    in1=r4.broadcast_to((128, nb, 4, 32)),
                                    op=O.mult)
            nc.vector.tensor_tensor(out=s[:, :nb, :], in0=s[:, :nb, :],
                                    in1=fo[:, :nb, :], op=O.add)

            if layer == 0:
                # y = elu(elu(s)) ; elu(x) = relu(x) + exp(min(x,0)) - 1
                sv = s[:, :nb, :]
                m = fo   # residual tile is dead now; reuse as scratch
                mv = m[:, :nb, :]
                nc.vector.tensor_scalar_min(out=mv, in0=sv, scalar1=0.0)
                em = fpool.tile([128, HB, 128], dt.float32, tag="em")
                emv = em[:, :nb, :]
                nc.scalar.activation(emv, mv, A.Exp)
                y1 = fpool.tile([128, HB, 128], dt.float32, tag="y1")
                y1v = y1[:, :nb, :]
                nc.vector.scalar_tensor_tensor(out=y1v, in0=sv, scalar=0.0,
                                               in1=emv, op0=O.max, op1=O.add)
                nc.vector.tensor_scalar_min(out=mv, in0=y1v, scalar1=1.0)
                nc.scalar.activation(emv, mv, A.Exp, bias=negone[:])
                nc.vector.tensor_scalar_max(out=y1v, in0=y1v, scalar1=1.0)
                nc.vector.scalar_tensor_tensor(out=sv, in0=y1v,
                                               scalar=-2.0, in1=emv,
                                               op0=O.add, op1=O.add)
                ost = s
                ow = 128
            else:
                ost = fpool.tile([128, HB, OUT], dt.float32, tag="om")
                nc.vector.tensor_tensor(out=ost[:, :nb, :], in0=s[:, :nb, 0:32],
                                        in1=s[:, :nb, 32:64], op=O.add)
                h23 = fpool.tile([128, HB, OUT], dt.float32, tag="h23")
                nc.vector.tensor_tensor(out=h23[:, :nb, :], in0=s[:, :nb, 64:96],
                                        in1=s[:, :nb, 96:128], op=O.add)
                nc.vector.tensor_tensor(out=ost[:, :nb, :], in0=ost[:, :nb, :],
                                        in1=h23[:, :nb, :], op=O.add)
                nc.vector.tensor_scalar_mul(out=ost[:, :nb, :],
                                            in0=ost[:, :nb, :], scalar1=0.25)
                ow = OUT

            nc.sync.dma_start(outv[:, kb0:kb1, :], ost[:, :nb, :ow])

    nc.compile()
    return nc


# ---------------------------------------------------------------- runner ----
def make_runner(nc, n_cores, use_donate=False):
    install_neuronx_cc_hook()
    partition_name = nc.partition_id_tensor.name if nc.partition_id_tensor else None
    in_names, out_names, out_avals, zero_outs = [], [], [], []
    for alloc in nc.m.functions[0].allocations:
        if not isinstance(alloc, mybir.MemoryLocationSet):
            continue
        name = alloc.memorylocations[0].name
        if alloc.kind == "ExternalInput":
            if name != partition_name:
                in_names.append(name)
        elif alloc.kind == "ExternalOutput":
            dtp = mybir.dt.np(alloc.dtype)
            out_avals.append(jax.core.ShapedArray(tuple(alloc.tensor_shape), dtp))
            out_names.append(name)
            zero_outs.append(np.zeros(tuple(alloc.tensor_shape), dtp))
    n_params = len(in_names)
    n_outs = len(out_names)
    in_names.extend(out_names)
    if partition_name is not None:
        in_names.append(partition_name)
    donate = tuple(range(n_params, n_params + n_outs))

    def _body(*args):
        operands = list(args)
        if partition_name is not None:
            operands.append(bass2jax.partition_id_tensor())
        outs = _bass_exec_p.bind(
            *operands, out_avals=tuple(out_avals), in_names=tuple(in_names),
            out_names=tuple(out_names), lowering_input_output_aliases=(),
            sim_require_finite=True, sim_require_nnan=True, nc=nc)
        return tuple(outs)

    devices = jax.devices()[:n_cores]
    mesh = Mesh(np.asarray(devices), ("core",))
    sharded = jax.jit(
        shard_map(_body, mesh=mesh,
                  in_specs=(PartitionSpec("core"),) * (n_params + n_outs),
                  out_specs=(PartitionSpec("core"),) * n_outs,
                  check_rep=False),
        donate_argnums=(donate if use_donate else ()), keep_unused=True)

    class Runner:
        def __init__(self):
            self.in_names = in_names; self.out_names = out_names
            self.real_in_names = in_names[:n_params]
            self.out_avals = out_avals; self.n_cores = n_cores
        def prep(self, in_maps):
            concat = [np.concatenate([m[nm] for m in in_maps], axis=0) for nm in self.real_in_names]
            concat += [np.concatenate([z]*n_cores, axis=0) for z in zero_outs]
            sh = NamedSharding(mesh, PartitionSpec("core"))
            return [jax.device_put(a, sh) for a in concat]
        def run(self, dev_args):
            return sharded(*dev_args)
        def run_np(self, in_maps):
            outs = self.run(self.prep(in_maps))
            return [
                {nm: np.asarray(outs[i]).reshape(n_cores, *out_avals[i].shape)[c]
                 for i, nm in enumerate(out_names)}
                for c in range(n_cores)]
        def time_steady(self, dev_args, iters=6, warmup=2):
            for _ in range(warmup):
                jax.block_until_ready(self.run(dev_args))
            ts = []
            for _ in range(iters):
                t0 = time.perf_counter()
                jax.block_until_ready(self.run(dev_args))
                ts.append(time.perf_counter() - t0)
            return min(ts), ts
    return Runner()


# ------------------------------------------------------------- host glue ----
def make_consts(attn):
    """attnA [128, 4]: block-diagonal attention vectors; ident; iota."""
    attnA = np.zeros((128, 4), f32)
    for h in range(HEADS):
        attnA[h * OUT:(h + 1) * OUT, h] = attn[h]
    ident = np.eye(128, dtype=f32)
    iota = np.tile(np.arange(128, dtype=f32)[None, :], (128, 1))
    return attnA.astype(bf16), ident.astype(bf16), iota.astype(bf16)


def layer_in_maps(plans, featsT_bf, feats_full, W_src, W_dst, b_src, b_dst,
                  attn, bias):
    """Build per-core in_maps for one layer launch (tables projected on host)."""
    attnA, ident, iota = make_consts(np.asarray(attn, f32))
    bc = (np.asarray(b_src, f32) + np.asarray(b_dst, f32)).reshape(1, -1)
    fo_extra = (np.asarray(bias, f32) + np.asarray(b_src, f32)).reshape(1, -1)
    ff = feats_full.astype(f32)
    fs_all = ff @ np.asarray(W_src, f32)              # [N, 128] unbiased
    fd_all = ff @ np.asarray(W_dst, f32) + bc         # [N, 128] biased
    rows_src = (np.arange(N) % 128) * NCH_SRC + np.arange(N) // 128
    tbl_src = np.zeros((NPAD_SRC, 128), bf16)
    tbl_src[rows_src] = fs_all.astype(bf16)
    rows_dst = (np.arange(OWN) % 128) * NCH_DST + np.arange(OWN) // 128
    in_maps = []
    for c, p in enumerate(plans):
        base = c * OWN
        def wrap16rep(a):
            return np.tile(a.reshape(-1, 16).T, (8, 1)).copy()
        fo = np.zeros((NPAD_DST, 128), f32)
        fo[:OWN] = feats_full[base:base + OWN].astype(f32) + fo_extra
        # permute rows: r = p*NBLK + c holds own node c*128 + p
        fo = np.ascontiguousarray(
            fo.reshape(NBLK, 128, 128).transpose(1, 0, 2).reshape(NPAD_DST, 128))
        tbl_dst = np.zeros((NPAD_DST, 128), bf16)
        tbl_dst[rows_dst] = fd_all[base:base + OWN].astype(bf16)
        in_maps.append(dict(
            tbl_src=tbl_src, tbl_dst=tbl_dst, fo=fo,
            ident=ident, attnA=attnA, iota=iota,
            dstloc2=p["dstloc2"],
            idx_src=wrap16rep(p["idx_src"]),
        ))
    return in_maps


def unpermute_out(o):
    """[NPAD_DST, w] permuted (p-major) -> [OWN, w] natural order."""
    w = o.shape[1]
    return o.reshape(128, NBLK, w).transpose(1, 0, 2).reshape(NPAD_DST, w)[:OWN]


class TwoLayerRunner:
    def __init__(self, src, dst, verbose=False):
        self.plans = make_plans(src, dst)
        self.T = self.plans[0]["T"]
        if verbose:
            print(f"common T={self.T} tiles ({self.T*128} idx slots)")
        self.nc0 = build_layer(self.plans[0], layer=0)
        self.nc1 = build_layer(self.plans[0], layer=1)
        self.r0 = make_runner(self.nc0, N_CORES)
        self.r1 = make_runner(self.nc1, N_CORES)

    def __call__(self, feats, inp):
        featsT_bf = np.ascontiguousarray(feats.T).astype(bf16)
        m0 = layer_in_maps(self.plans, featsT_bf, feats,
                           inp["W_src0"], inp["W_dst0"], inp["b_src0"][:],
                           inp["b_dst0"][:], inp["attn0"], inp["bias0"])
        outs0 = self.r0.run_np(m0)
        h1 = np.concatenate([unpermute_out(o["out_own"]) for o in outs0],
                            axis=0)  # [N, 128]
        h1T_bf = np.ascontiguousarray(h1.T).astype(bf16)
        m1 = layer_in_maps(self.plans, h1T_bf, h1,
                           inp["W_src1"], inp["W_dst1"], inp["b_src1"][:],
                           inp["b_dst1"][:], inp["attn1"], inp["bias1"])
        outs1 = self.r1.run_np(m1)
        out = np.concatenate([unpermute_out(o["out_own"]) for o in outs1],
                             axis=0)  # [N, 32]
        return h1, out


_TLR_CACHE = {}


def kernel(**inputs):
    inputs = {k: np.asarray(v) for k, v in inputs.items()}
    src = inputs["src"].astype(np.int64)
    dst = inputs["dst"].astype(np.int64)
    feats = inputs["feats"].astype(np.float32)
    kh = hash((src.tobytes(), dst.tobytes()))
    if kh not in _TLR_CACHE:
        _TLR_CACHE[kh] = TwoLayerRunner(src, dst)
    tlr = _TLR_CACHE[kh]
    _h1, out = tlr(feats, inputs)
    return out.astype(np.float32)


_NULL_CACHE = {}


def null_baseline():
    """Steady-state wall of a near-empty 8-core launch (dispatch overhead)."""
    if "t" in _NULL_CACHE:
        return _NULL_CACHE["t"]
    nc = bacc.Bacc("TRN2", target_bir_lowering=False, debug=False,
                   num_devices=N_CORES)
    x = nc.dram_tensor("x", (128, 128), dt.float32, kind="ExternalInput")
    y = nc.dram_tensor("y", (128, 128), dt.float32, kind="ExternalOutput")
    with tile.TileContext(nc) as tc, ExitStack() as ctx:
        pool = ctx.enter_context(tc.tile_pool(name="sbuf", bufs=2))
        t = pool.tile([128, 128], dt.float32)
        nc.sync.dma_start(t[:], x[:])
        nc.sync.dma_start(y[:], t[:])
    nc.compile()
    r = make_runner(nc, N_CORES)
    xs = np.zeros((128, 128), np.float32)
    dev = r.prep([{"x": xs}] * N_CORES)
    best, _ = r.time_steady(dev, iters=30, warmup=3)
    _NULL_CACHE["t"] = best
    return best
